# revision 19
# baseline (speedup 1.0000x reference)
"""Trainium2 Bass kernel for a full-attention MHA layer (B=2, S=2048, HID=2048,
16 heads, head_dim=128, RoPE, no mask), sharded over 8 NeuronCores as
2 batches x 4 head-groups (4 heads per core).

Per-core dataflow (feature-major, so no probability transposes are needed):
  hT       = host-pretransposed hidden                 [d, t] bf16 (from DRAM)
  qT,kT    = w_qkvT.T @ hT                             [d, t] per head + RoPE
  v        = hT.T @ w_vT                               [t, d] natural layout
  ST       = kT.T @ qT                                 [tk, tq] scores transposed
  PT       = exp(ST * scale)    (ACT over PAIRS of PSUM banks, fused fp32->bf16)
  den      = onesT.T @ PT                              partition sum, replicated
  OT       = (v.T @ PT) * (1/den)                      [d, tq]
  OUT      = OT.T @ w_oT                               [t, o] bf16 partial

Host pre-transposes hidden to feature-major (so no on-device transposes) and
sums the 4 per-batch bf16 partial OUTs in fp32.
"""
import numpy as np
import ml_dtypes

import concourse.bass as bass
import concourse.mybir as mybir
from concourse import bacc, tile

B, S, HID = 2, 2048, 2048
NH, HD = 16, 128
G = 4                 # head-groups = cores per batch
NHL = NH // G         # heads per core
KO = HID // 128       # 16 contraction chunks
TS = 512              # token slice for the projection phase
NSL = S // TS         # 4
TQ = 512              # query-tile width in attention
NTQ = S // TQ         # 4
NTK = S // 128        # 16 key chunks
FQK = NHL * HD        # 512 features for q (and k) per core
FV = NHL * HD         # 512 features for v per core
BF16 = mybir.dt.bfloat16
F32 = mybir.dt.float32
SCALE = 1.0 / float(np.sqrt(HD))

N_CORES = 8


def _emit(nc, tc, hid, wq, wo, cosT, ssinT, outp, repeats=1):
    from contextlib import ExitStack
    ctx = ExitStack()
    with ctx:
        const = ctx.enter_context(tc.tile_pool(name="const", bufs=1))
        persist = ctx.enter_context(tc.tile_pool(name="persist", bufs=1))
        work = ctx.enter_context(tc.tile_pool(name="work", bufs=2))
        small = ctx.enter_context(tc.tile_pool(name="small", bufs=2))
        psA = ctx.enter_context(tc.tile_pool(name="psA", bufs=2, space="PSUM"))
        psB = ctx.enter_context(tc.tile_pool(name="psB", bufs=2, space="PSUM"))
        psC = ctx.enter_context(tc.tile_pool(name="psC", bufs=2, space="PSUM"))

        # ---- constants ----
        # wq q/k chunks stream on the scalar HWDGE ring (interleaving with the
        # slice-0 hT chunks on the sync ring) so the first projection matmul
        # can start ~2us in; everything else rides the SWDGE ring.
        ones_sb = const.tile([128, 128], BF16)
        nc.vector.memset(ones_sb, 1.0)
        # qk chunk k is consumed at ~2+1.3k us, v chunk k from ~22+0.9k us:
        # emit qk 0-9 first, then interleave the qk tail with the v head so
        # both streams arrive just ahead of their consumers.
        wq_full = const.tile([128, KO, 3 * FQK], BF16)
        qk_order = [("qk", k) for k in range(10)]
        for i in range(6):
            qk_order += [("qk", 10 + i), ("v", i)]
        qk_order += [("v", k) for k in range(6, KO)]
        for kind, ko in qk_order:
            if kind == "qk":
                nc.scalar.dma_start(wq_full[:, ko, 0:2 * FQK], wq[:, ko, 0:2 * FQK])
            else:
                nc.scalar.dma_start(wq_full[:, ko, 2 * FQK:], wq[:, ko, 2 * FQK:])
        # cos/ssin ride the scalar ring behind the wq chunks: they are first
        # needed ~20us in, and keeping them out of the first ~6MB of HBM
        # traffic keeps the projection stream DMA-paced.
        # cos/ssin/wo queue on the scalar ring BEHIND the wq chunks (the ring
        # executes in emission order): they are first needed ~20us/~190us in,
        # and keeping them out of the first ~6MB of HBM traffic keeps the
        # startup projection stream DMA-paced.
        cos_sb = const.tile([128, S], BF16)
        nc.scalar.dma_start(cos_sb, cosT)
        ssin_sb = const.tile([128, S], BF16)
        nc.scalar.dma_start(ssin_sb, ssinT)
        wo_sb = const.tile([128, NHL, HID], BF16)
        nc.scalar.dma_start(wo_sb, wo)

        for _rep in range(repeats):
            _emit_body(nc, tc, hid, outp, wq_full, wo_sb, cos_sb, ssin_sb,
                       ones_sb, persist, work, small, psA, psB, psC)


def _emit_body(nc, tc, hid, outp, wq_sb, wo_sb, cos_sb, ssin_sb,
               ones_sb, persist, work, small, psA, psB, psC):
        # ---- persistent activations ----
        qT = persist.tile([128, NHL, S], BF16, tag="qT", bufs=1)   # [d, h, t]
        kT = persist.tile([128, NHL, S], BF16, tag="kT", bufs=1)   # [d, h, t]
        vN = persist.tile([128, NTK, FV], BF16, tag="vN", bufs=1)  # [t%128, t//128, f]
        oT = persist.tile([128, NHL, S], BF16, tag="oT", bufs=1)   # [d, h, tq]

        # ======== Phase A: QKV projections + RoPE ========
        # 12 groups per slice (0-3 q heads, 4-7 k heads, 8-11 v token-chunks),
        # processed ko-outer in halves of 6 PSUM banks (borrowing the psB/psC
        # banks that are idle until attention) so slice 0 streams against the
        # arriving wq chunks and half-boundaries never wait on PSUM drains.
        def rope_store(grp, ps1, t0):
            if grp < NHL:
                dest = qT[:, grp, t0:t0 + TS]
            else:
                dest = kT[:, grp - NHL, t0:t0 + TS]
            nc.vector.tensor_copy(dest, ps1)
            return dest

        def rope_apply(dest, t0):
            # dest = dest*cos + swap(dest)*ssin  (sign folded into ssin)
            sw = small.tile([128, TS], BF16, tag="sw", bufs=4)
            nc.gpsimd.dma_start(sw[0:64, :], dest[64:128, :])
            nc.gpsimd.dma_start(sw[64:128, :], dest[0:64, :])
            nc.vector.tensor_mul(dest, dest, cos_sb[:, t0:t0 + TS])
            nc.vector.tensor_mul(sw, sw, ssin_sb[:, t0:t0 + TS])
            nc.vector.tensor_add(dest, dest, sw)

        for ts_i in range(NSL):
            t0 = ts_i * TS
            hT = work.tile([128, KO, TS], BF16, tag="hT", name="hT")
            if ts_i == 0:
                for ko in range(KO):
                    nc.sync.dma_start(hT[:, ko, :], hid[:, 0, ko, :])
            else:
                nc.sync.dma_start(hT, hid[:, ts_i])
            for half in range(2):
                if half == 0:
                    pdA = [psA.tile([128, 2, 512], F32, tag="mm", name=f"pdA{i}")
                           for i in range(2)]
                    pb = [psB.tile([128, TQ], F32, tag="acc", name=f"pb{i}")
                          for i in range(2)]
                    slots = [pdA[0][:, 0, :], pdA[0][:, 1, :],
                             pdA[1][:, 0, :], pdA[1][:, 1, :], pb[0], pb[1]]
                elif ts_i < NSL - 1:
                    pc = [psC.tile([128, 512], F32, tag="out", name=f"pc{i}")
                          for i in range(2)]
                    pdB = [psA.tile([128, 2, 512], F32, tag="mm", name=f"pdB{i}")
                           for i in range(2)]
                    slots = [pc[0], pc[1], pdB[0][:, 0, :], pdB[0][:, 1, :],
                             pdB[1][:, 0, :], pdB[1][:, 1, :]]
                else:
                    # last slice: keep the v groups out of psA so the first
                    # attention score pair doesn't wait on their PSUM drain;
                    # k2/k3 (drained first) take the lone psA double.
                    pdB = [psA.tile([128, 2, 512], F32, tag="mm", name="pdB0")]
                    pb = [psB.tile([128, TQ], F32, tag="acc", name=f"pb{i}")
                          for i in range(2)]
                    pc = [psC.tile([128, 512], F32, tag="out", name=f"pc{i}")
                          for i in range(2)]
                    slots = [pdB[0][:, 0, :], pdB[0][:, 1, :],
                             pb[0], pb[1], pc[0], pc[1]]
                for ko in range(KO):
                    for g in range(6):
                        grp = half * 6 + g
                        ps1 = slots[g]
                        if grp < 8:
                            nc.tensor.matmul(ps1,
                                             wq_sb[:, ko, grp * 128:(grp + 1) * 128],
                                             hT[:, ko, :],
                                             start=(ko == 0), stop=(ko == KO - 1))
                        else:
                            tt = grp - 8
                            nc.tensor.matmul(ps1,
                                             hT[:, ko, tt * 128:(tt + 1) * 128],
                                             wq_sb[:, ko, 2 * FQK:3 * FQK],
                                             start=(ko == 0), stop=(ko == KO - 1))
                # In the last slice, drain+RoPE the k heads first so the first
                # attention scores don't wait behind the q RoPE / v copies on
                # the DVE FIFO.
                order = list(range(6))
                if ts_i == NSL - 1:
                    order.sort(key=lambda g: 0 if NHL <= half * 6 + g < 2 * NHL else 1)
                dests = []
                for g in order:
                    grp = half * 6 + g
                    if grp < 8:
                        dests.append(rope_store(grp, slots[g], t0))
                        if ts_i == NSL - 1 and grp >= NHL:
                            rope_apply(dests.pop(), t0)
                    else:
                        nc.vector.tensor_copy(vN[:, ts_i * 4 + (grp - 8), :], slots[g])
                for dest in dests:
                    rope_apply(dest, t0)
        # ======== Phase C+D: attention, with the previous tile's out-proj
        # interleaved into each head's exp-pipeline-fill window ========
        def emit_og(tt, ot, last_tile):
            ps = psC.tile([128, 512], F32, tag="out", name="og")
            for h2 in range(NHL):
                nc.tensor.matmul(ps,
                                 oT[:, h2, tt * 128:(tt + 1) * 128],
                                 wo_sb[:, h2, ot * 512:(ot + 1) * 512],
                                 start=(h2 == 0), stop=(h2 == NHL - 1))
            ob = small.tile([128, 512], BF16, tag="ob", bufs=6)
            # in the final (uncovered) tile, spread the drain across both copy
            # engines and both DMA rings so the kernel tail isn't serialized.
            if last_tile and (tt + ot) % 2 == 1:
                nc.scalar.copy(ob, ps)
                nc.gpsimd.dma_start(outp[tt * 128:(tt + 1) * 128,
                                         ot * 512:(ot + 1) * 512], ob)
            else:
                nc.vector.tensor_copy(ob, ps)
                nc.sync.dma_start(outp[tt * 128:(tt + 1) * 128,
                                       ot * 512:(ot + 1) * 512], ob)

        pending_og = []
        for tqi in range(NTQ):
            tq0 = tqi * TQ
            for h in range(NHL):
                den = psB.tile([128, TQ], F32, tag="acc")
                pv = psB.tile([128, TQ], F32, tag="acc")

                def emit_score_pair(p):
                    psd = psA.tile([128, 2, 512], F32, tag="mm", name="psd")
                    for j in range(2):
                        nc.tensor.matmul(psd[:, j, :TQ],
                                         kT[:, h, (2 * p + j) * 128:(2 * p + j + 1) * 128],
                                         qT[:, h, tq0:tq0 + TQ],
                                         start=True, stop=True)
                    pt = small.tile([128, 2, TQ], BF16, tag="pt", bufs=4, name="pt")
                    nc.scalar.activation(pt, psd,
                                         mybir.ActivationFunctionType.Exp,
                                         scale=SCALE)
                    return pt

                # software pipeline: score pairs run 2 ahead of the PV matmuls;
                # the den reduction collapses each group of 4 prob chunks to one
                # matmul via bf16 pair-sums on DVE.  The den matmul for group g
                # is deferred into group g+1's PV window so it never stalls on
                # the DVE add chain.
                pts = [emit_score_pair(0), emit_score_pair(1)]
                for _ in range(4):
                    if pending_og:
                        emit_og(*pending_og.pop(0), last_tile=False)
                den_pending = None
                for grp in range(NTK // 4):
                    p0, p1 = pts
                    # pair-sums up front: they only need the exps, and doing
                    # them early keeps the (deferred) den matmuls off the DVE
                    # critical path.
                    s1 = small.tile([128, TQ], BF16, tag="ptsum", bufs=3, name="s1")
                    nc.vector.tensor_add(s1, p0[:, 0, :], p0[:, 1, :])
                    s2 = small.tile([128, TQ], BF16, tag="ptsum", bufs=3, name="s2")
                    nc.vector.tensor_add(s2, p1[:, 0, :], p1[:, 1, :])
                    nc.vector.tensor_add(s1, s1, s2)
                    for j in range(4):
                        src = p0 if j < 2 else p1
                        nc.tensor.matmul(pv,
                                         vN[:, 4 * grp + j, h * HD:(h + 1) * HD],
                                         src[:, j % 2, :],
                                         start=(grp == 0 and j == 0),
                                         stop=(grp == NTK // 4 - 1 and j == 3))
                        if grp < NTK // 4 - 1 and j in (1, 3):
                            pts[j // 2] = emit_score_pair(2 * grp + 2 + j // 2)
                        if j == 1 and den_pending is not None:
                            nc.tensor.matmul(den, ones_sb, den_pending,
                                             start=(grp == 1), stop=False)
                            den_pending = None
                    den_pending = s1
                nc.tensor.matmul(den, ones_sb, den_pending,
                                 start=False, stop=True)
                rec = small.tile([128, TQ], F32, tag="rec", bufs=2)
                nc.vector.reciprocal(rec, den)
                nc.vector.tensor_mul(oT[:, h, tq0:tq0 + TQ], pv, rec)
            pending_og = [(tt, ot)
                          for tt in range(tqi * (TQ // 128), (tqi + 1) * (TQ // 128))
                          for ot in range(HID // 512)]
        for tt, ot in pending_og:
            emit_og(tt, ot, last_tile=True)


def build(repeats=1):
    nc = bacc.Bacc("TRN2", target_bir_lowering=False, debug=False)
    hid = nc.dram_tensor("hid", [128, NSL, KO, TS], BF16, kind="ExternalInput")
    wq = nc.dram_tensor("wq", [128, KO, 3 * FQK], BF16, kind="ExternalInput")
    wo = nc.dram_tensor("wo", [128, NHL, HID], BF16, kind="ExternalInput")
    cosT = nc.dram_tensor("cosT", [128, S], BF16, kind="ExternalInput")
    ssinT = nc.dram_tensor("ssinT", [128, S], BF16, kind="ExternalInput")
    outp = nc.dram_tensor("outp", [S, HID], BF16, kind="ExternalOutput")
    with tile.TileContext(nc) as tc:
        _emit(nc, tc, hid.ap(), wq.ap(), wo.ap(), cosT.ap(), ssinT.ap(), outp.ap(),
              repeats=repeats)
    nc.compile()
    return nc


def shard_inputs(hidden_states, cos, sin, w_qkv, w_o):
    """Build the 8 per-core input maps (host-side layout prep)."""
    hidden_states = np.asarray(hidden_states, dtype=np.float32)
    cos = np.asarray(cos, dtype=np.float32)
    sin = np.asarray(sin, dtype=np.float32)
    w_qkv = np.asarray(w_qkv, dtype=np.float32)
    w_o = np.asarray(w_o, dtype=np.float32)

    cosT = np.ascontiguousarray(cos[:, 0, :].T).astype(ml_dtypes.bfloat16)
    sT = sin[:, 0, :].T.copy()
    sT[:64] = -sT[:64]
    ssinT = np.ascontiguousarray(sT).astype(ml_dtypes.bfloat16)

    # feature-major hidden: hidT[p, sl, ko, t] = hidden[b][sl*TS+t, ko*128+p]
    hidT = [np.ascontiguousarray(
        hidden_states[b].T.reshape(KO, 128, NSL, TS).transpose(1, 2, 0, 3)
    ).astype(ml_dtypes.bfloat16) for b in range(B)]

    woT = w_o.T  # [j, o]
    in_maps = []
    for c in range(N_CORES):
        b, g = divmod(c, G)
        rows = np.concatenate([
            w_qkv[FQK * g: FQK * (g + 1)],
            w_qkv[NH * HD + FQK * g: NH * HD + FQK * (g + 1)],
            w_qkv[2 * NH * HD + FQK * g: 2 * NH * HD + FQK * (g + 1)],
        ], axis=0)                                   # [1536, 2048]
        wq_pack = np.ascontiguousarray(
            rows.T.reshape(KO, 128, 3 * FQK).transpose(1, 0, 2)
        ).astype(ml_dtypes.bfloat16)                 # [128, KO, 1536]
        wo_pack = np.ascontiguousarray(
            woT[FQK * g: FQK * (g + 1)].reshape(NHL, 128, HID).transpose(1, 0, 2)
        ).astype(ml_dtypes.bfloat16)                 # [128, NHL, 2048]
        in_maps.append({
            "hid": hidT[b],
            "wq": wq_pack,
            "wo": wo_pack,
            "cosT": cosT,
            "ssinT": ssinT,
        })
    return in_maps


def gather_outputs(results):
    """results: list of 8 dicts with 'outp' -> full [B, S, HID] output."""
    out = np.zeros((B, S, HID), dtype=np.float32)
    for c in range(N_CORES):
        b = c // G
        out[b] += results[c]["outp"].astype(np.float32)
    return out


# ---------------- cached runner over PJRT/axon ----------------
_RUNNER = None


def _make_runner():
    import jax
    from jax.sharding import Mesh, PartitionSpec, NamedSharding
    from jax.experimental.shard_map import shard_map
    from concourse import bass2jax

    nc = build()
    bass2jax.install_neuronx_cc_hook()
    partition_name = nc.partition_id_tensor.name if nc.partition_id_tensor else None
    in_names, out_names, out_avals = [], [], []
    for alloc in nc.m.functions[0].allocations:
        if not isinstance(alloc, mybir.MemoryLocationSet):
            continue
        name = alloc.memorylocations[0].name
        if alloc.kind == "ExternalInput":
            if name != partition_name:
                in_names.append(name)
        elif alloc.kind == "ExternalOutput":
            out_names.append(name)
            out_avals.append(jax.core.ShapedArray(
                tuple(alloc.tensor_shape), mybir.dt.np(alloc.dtype)))
    n_params = len(in_names)
    all_in_names = list(in_names) + list(out_names)
    if partition_name is not None:
        all_in_names.append(partition_name)

    import hashlib
    import os as _os
    _tag = hashlib.sha256(open(__file__, "rb").read()
                          + _os.environ.get("BASS_KERNEL_TAG", "").encode()).hexdigest()[:12]

    def _body(*args):
        operands = list(args)
        if partition_name is not None:
            operands.append(bass2jax.partition_id_tensor())
        outs = bass2jax._bass_exec_p.bind(
            *operands,
            out_avals=tuple(out_avals),
            in_names=tuple(all_in_names),
            out_names=tuple(out_names),
            lowering_input_output_aliases=(),
            sim_require_finite=True,
            sim_require_nnan=True,
            nc=nc,
        )
        return tuple(outs)

    devices = jax.devices()[:N_CORES]
    mesh = Mesh(np.asarray(devices), ("core",))
    n_outs = len(out_names)
    in_specs = (PartitionSpec("core"),) * (n_params + n_outs)
    out_specs = (PartitionSpec("core"),) * n_outs
    donate = tuple(range(n_params, n_params + n_outs))
    _body.__name__ = f"body_{_tag}"
    _sharded = shard_map(_body, mesh=mesh, in_specs=in_specs, out_specs=out_specs,
                         check_rep=False)

    def _entry(*args):
        return _sharded(*args)
    _entry.__name__ = f"bass_attn_{_tag}"
    fn = jax.jit(_entry, donate_argnums=donate, keep_unused=True)
    sharding = NamedSharding(mesh, PartitionSpec("core"))

    class Runner:
        def __init__(self):
            self.fn = fn
            self.nc = nc
            self.in_names = in_names
            self.out_names = out_names
            self.out_avals = out_avals
            self.sharding = sharding

        def stage(self, in_maps):
            import jax
            concat = [np.concatenate([in_maps[c][n] for c in range(N_CORES)], axis=0)
                      for n in self.in_names]
            return [jax.device_put(x, self.sharding) for x in concat]

        def zeros(self):
            import jax
            import jax.numpy as jnp
            if not hasattr(self, "_zeros_fn"):
                shapes = [((N_CORES * av.shape[0],) + tuple(av.shape[1:]), av.dtype)
                          for av in self.out_avals]
                self._zeros_fn = jax.jit(
                    lambda: tuple(jnp.zeros(s, d) for s, d in shapes),
                    out_shardings=tuple(self.sharding for _ in shapes))
            return list(self._zeros_fn())

        def run(self, dev_in, outs=None):
            if outs is None:
                outs = self.zeros()
            return self.fn(*dev_in, *outs)

        def split(self, outs):
            import jax
            jax.block_until_ready(outs)
            res = []
            for c in range(N_CORES):
                res.append({
                    n: np.asarray(outs[i]).reshape(
                        N_CORES, *self.out_avals[i].shape)[c]
                    for i, n in enumerate(self.out_names)})
            return res

    return Runner()


def get_runner():
    global _RUNNER
    if _RUNNER is None:
        _RUNNER = _make_runner()
    return _RUNNER


def kernel(hidden_states, cos, sin, w_qkv, w_o):
    r = get_runner()
    in_maps = shard_inputs(hidden_states, cos, sin, w_qkv, w_o)
    dev_in = r.stage(in_maps)
    outs = r.run(dev_in)
    results = r.split(outs)
    return gather_outputs(results)
